# revision 26
# baseline (speedup 1.0000x reference)
"""Distributed Trainium2 kernel for nn_Attention_37958920962105.

GQA attention layer (DIM=4096, 32 q heads, 8 kv heads, head_dim=128,
B=2, S=2048) sharded tensor-parallel over GQA groups across 8 cores:
core c owns q heads 4c..4c+3 and kv head c.  Per core:
  1. QKV projection (transposed layouts) + RoPE (pair-swap via strided
     SBUF->SBUF DMA) + v transpose (XBAR DMA transpose)
  2. attention (scores -> exp -> paired-denominator matmuls -> AV)
  3. AllToAll (one per local head) to token-shard y
  4. out projection on this core's 512-token chunk (first two output
     tiles run their m<=2 chunks first so the last AllToAll hides)
Output chunks are reassembled on the host.
"""

import math
import sys
import types
from contextlib import ExitStack

import numpy as np
import ml_dtypes

import concourse.bass as bass
import concourse.mybir as mybir
import concourse.tile as tile
from concourse import bacc
from concourse.bass_utils import run_bass_kernel_spmd

BF = mybir.dt.bfloat16
F32 = mybir.dt.float32
bf16 = ml_dtypes.bfloat16

P = 128
DIM = 4096
N_HEAD = 32
N_KV = 8
HD = 128
B = 2
S = 2048
TOK = B * S          # 4096
NCORES = 8
HPC = N_HEAD // N_KV  # 4 q heads per core
FQKV = (HPC + 2) * HD  # 768 qkv rows per core
KC = DIM // P        # 32 contraction chunks
KCB = 8              # kc chunks per batched DMA
N_TT = TOK // 512    # 8 token tiles of 512
QT_N = S // 512      # 4 q tiles per batch
KT_N = S // P        # 16 k tiles per batch
SCALE = 1.0 / math.sqrt(HD)


def _install_profile_shim():
    if 'antenv.axon_hooks' in sys.modules:
        return
    try:
        from trn_agent_boot.trn_boot import _ntff_profile_via_ctypes
        hook = _ntff_profile_via_ctypes('/opt/axon/libaxon_pjrt.so')
    except Exception:
        hook = None
    mod = types.ModuleType('antenv.axon_hooks')
    mod._hook = hook
    mod.get_axon_ntff_profile_hook = lambda: mod._hook
    mod.set_axon_ntff_profile_hook = lambda h: setattr(mod, '_hook', h)
    sys.modules['antenv.axon_hooks'] = mod
    try:
        import antenv
        antenv.axon_hooks = mod
    except ImportError:
        pass


# ---------------------------------------------------------------------------
# host-side prep
# ---------------------------------------------------------------------------

def _classify_mask(mask):
    """mask: [S(q), S(k)] bool.  Returns (cls, mask_tiles) where
    cls[qt][kt] in {'skip', 'free', int mask-tile-index}; mask tiles are
    transposed [128 k, 512 q] bf16."""
    cls = [[None] * KT_N for _ in range(QT_N)]
    tiles = []
    seen = {}
    qi = np.arange(512)[:, None]
    ki = np.arange(P)[None, :]
    for qt in range(QT_N):
        for kt in range(KT_N):
            blk = mask[qt * 512:(qt + 1) * 512, kt * P:(kt + 1) * P]
            if not blk.any():
                cls[qt][kt] = 'skip'
                continue
            if blk.all():
                cls[qt][kt] = 'free'
                continue
            off = kt * P - qt * 512
            if 0 <= off < 512 and np.array_equal(blk, (off + ki) <= qi):
                # causal diagonal tile: valid only for q >= off, and within
                # the first 128 valid q columns it is the k<=q' triangle
                cls[qt][kt] = ('diag', off)
                continue
            key = blk.tobytes()
            if key not in seen:
                seen[key] = len(tiles)
                tiles.append(np.ascontiguousarray(blk.T).astype(bf16))
            cls[qt][kt] = seen[key]
    return cls, tiles


def _prep(x, freqs_cis, mask_cache, wqkv, wo):
    x = np.asarray(x, dtype=np.float32)
    freqs_cis = np.asarray(freqs_cis, dtype=np.float32)
    wqkv = np.asarray(wqkv, dtype=np.float32)
    wo = np.asarray(wo, dtype=np.float32)
    mask = np.asarray(mask_cache)[0, 0]

    xT = np.ascontiguousarray(x.reshape(TOK, DIM).T).astype(bf16)

    wTs = []
    for c in range(NCORES):
        w_c = np.concatenate([
            wqkv[HPC * HD * c: HPC * HD * (c + 1)],          # 4 q heads
            wqkv[N_HEAD * HD + HD * c: N_HEAD * HD + HD * (c + 1)],   # k head
            wqkv[(N_HEAD + N_KV) * HD + HD * c:
                 (N_HEAD + N_KV) * HD + HD * (c + 1)],       # v head
        ], axis=0)                                           # [768, DIM]
        wTs.append(np.ascontiguousarray(w_c.T).astype(bf16))  # [DIM, 768]

    # wo permuted so row-block dbi = m*8 + cc holds global head 4*cc + m
    woT = np.ascontiguousarray(wo.T)                 # [d, o]
    woT_h = woT.reshape(N_HEAD, HD, DIM)
    perm = [4 * (dbi % NCORES) + dbi // NCORES for dbi in range(N_HEAD)]
    woT_perm = np.ascontiguousarray(woT_h[perm].reshape(DIM, DIM)).astype(bf16)

    f0 = freqs_cis[:, :, 0].T                        # [64, S]
    f1 = freqs_cis[:, :, 1].T
    ropeA = np.repeat(f0, 2, axis=0).astype(bf16)    # [128, S]
    ropeB = np.empty((HD, S), dtype=np.float32)
    ropeB[0::2] = -f1
    ropeB[1::2] = f1
    ropeB = ropeB.astype(bf16)

    ones_col = np.ones((P, 1), dtype=bf16)

    tri = (np.arange(P)[:, None] <= np.arange(P)[None, :]).astype(bf16)
    cls, mask_tiles = _classify_mask(mask)
    masks = (np.concatenate([t for t in mask_tiles], axis=0)
             if mask_tiles else None)                # [n*128, 512] bf16

    return dict(xT=xT, wTs=wTs, woT=woT_perm, ropeA=ropeA, ropeB=ropeB,
                ones=ones_col, tri=tri, cls=cls, masks=masks)


# ---------------------------------------------------------------------------
# device kernel builder
# ---------------------------------------------------------------------------

def _denom_pairs(kts, cls, qt):
    """Group the kt tiles of one (qt) into pairs for the denominator
    matmuls.  Returns a list of ('pair', i, j) / ('single', i) entries
    over indices into kts.  Free tiles are paired in order; the 4 diag
    tiles are paired (w512,w384) and (w256,w128)."""
    free = [i for i, kt in enumerate(kts) if cls[qt][kt] == 'free']
    diag = [i for i, kt in enumerate(kts) if isinstance(cls[qt][kt], tuple)]
    out = []
    for a in range(0, len(free) - 1, 2):
        out.append(('pair', free[a], free[a + 1]))
    if len(free) % 2:
        out.append(('single', free[-1]))
    # diag tiles sorted by offset ascending (w descending)
    diag.sort(key=lambda i: cls[qt][kts[i]][1])
    for a in range(0, len(diag) - 1, 2):
        out.append(('dpair', diag[a], diag[a + 1]))
    if len(diag) % 2:
        out.append(('single', diag[-1]))
    return out


def _dedupe_ldweights(nc):
    """Remove InstLdweights whose weights AP matches the previous PE
    weight load: the PE array already holds those weights, and skipping
    the reload removes the ~128-cycle drain-before-weight-swap stall
    between back-to-back matmuls.  Only drops loads with no semaphore
    waits/updates.  Safe here because every stationary operand in this
    kernel lives in a non-rotating SBUF tile (w_sb, kT, vv, ones, yf)."""
    n_removed = 0
    for fn in nc.m.functions:
        for bb in fn.blocks:
            lst = bb.instructions
            out = []
            prev_sig = None
            changed = False
            for inst in lst:
                nm = type(inst).__name__
                if nm == 'InstLdweights':
                    sig = (str(inst.ins[0]), str(inst.perf_mode),
                           str(inst.is_transpose), str(inst.tile_position))
                    if sig == prev_sig and inst.sync_info is None:
                        n_removed += 1
                        changed = True
                        continue
                    prev_sig = sig
                elif nm == 'InstMatmult':
                    if inst.is_transpose:
                        prev_sig = None
                elif nm in ('InstDrain', 'InstISA', 'InstCall',
                            'InstUnconditionalBranch'):
                    prev_sig = None
                out.append(inst)
            if changed:
                bb.instructions = out
    return n_removed


def _build(cls, n_masks, debug=False):
    nc = bacc.Bacc("TRN2", target_bir_lowering=False, debug=False,
                   num_devices=NCORES)
    xT_d = nc.dram_tensor("xT", [DIM, TOK], BF, kind="ExternalInput")
    wT_d = nc.dram_tensor("wT", [DIM, FQKV], BF, kind="ExternalInput")
    woT_d = nc.dram_tensor("woT", [DIM, DIM], BF, kind="ExternalInput")
    ropeA_d = nc.dram_tensor("ropeA", [P, S], BF, kind="ExternalInput")
    ropeB_d = nc.dram_tensor("ropeB", [P, S], BF, kind="ExternalInput")
    ones_d = nc.dram_tensor("ones", [P, 1], BF, kind="ExternalInput")
    tri_d = nc.dram_tensor("tri", [P, P], BF, kind="ExternalInput")
    masks_d = (nc.dram_tensor("masks", [n_masks * P, 512], BF,
                              kind="ExternalInput") if n_masks else None)
    out_d = nc.dram_tensor("out", [512, DIM], F32, kind="ExternalOutput")

    EXP = mybir.ActivationFunctionType.Exp
    rg = [list(range(NCORES))]

    with tile.TileContext(nc) as tc:
        with ExitStack() as top:
            const = top.enter_context(tc.tile_pool(name="const", bufs=1))
            actsA = top.enter_context(tc.tile_pool(name="actsA", bufs=1))
            actsB = top.enter_context(tc.tile_pool(name="actsB", bufs=1))
            dramp = top.enter_context(tc.tile_pool(name="dramp", bufs=1,
                                                   space="DRAM"))

            qT = [actsA.tile([P, TOK], BF, name=f"qT{h}") for h in range(HPC)]
            kT = actsA.tile([P, TOK], BF, name="kT")
            vv = [actsA.tile([P, P], BF, name=f"v{i}") for i in range(TOK // P)]

            a2a_in = [dramp.tile([NCORES * P, 512], BF, name=f"a2ai{m}")
                      for m in range(HPC)]
            a2a_out = [dramp.tile([NCORES * P, 512], BF, name=f"a2ao{m}")
                       for m in range(HPC)]

            # ---------------- phase 1: QKV + rope + v transpose ----------
            with ExitStack() as ph1, nc.named_scope("ph1_qkv"):
                wp = ph1.enter_context(tc.tile_pool(name="wp", bufs=1))
                xp = ph1.enter_context(tc.tile_pool(name="xp", bufs=6))
                w_src = wT_d[:].rearrange("(kc p) f -> kc p f", p=P)
                x_src = xT_d[:].rearrange("(kc p) t -> kc p t", p=P)

                def load_x_batch(xt, tt, kcb):
                    (nc.scalar if kcb % 2 else nc.sync).dma_start(
                        xt[:], xT_d[:, tt * 512:(tt + 1) * 512].rearrange(
                            "(kcb kc p) t -> kcb p kc t", p=P, kc=KCB)[kcb])

                # fine-grained interleaved first chunks so the very first
                # matmul can start after ~400 KiB instead of 2.5 MiB
                w0 = wp.tile([P, KCB, FQKV], BF, name="w0")
                x0 = xp.tile([P, KCB, 512], BF, name="xt")
                for kc in range(KCB):
                    (nc.sync if kc % 2 == 0 else nc.scalar).dma_start(
                        w0[:, kc, :], w_src[kc])
                    (nc.scalar if kc % 2 == 0 else nc.sync).dma_start(
                        x0[:, kc, :], x_src[kc, :, 0:512])
                w_sb = [w0]
                xts_cur = [x0]
                for kcb in range(1, KC // KCB):
                    w = wp.tile([P, KCB, FQKV], BF, name=f"w{kcb}")
                    (nc.gpsimd if kcb >= 2 else nc.scalar).dma_start(
                        w[:], wT_d[:].rearrange(
                            "(kcb kc p) f -> kcb p kc f", p=P, kc=KCB
                        )[kcb])
                    w_sb.append(w)
                    xt = xp.tile([P, KCB, 512], BF, name="xt")
                    load_x_batch(xt, 0, kcb)
                    xts_cur.append(xt)

                def load_x(tt):
                    tiles = []
                    for kcb in range(KC // KCB):
                        xt = xp.tile([P, KCB, 512], BF, name="xt")
                        load_x_batch(xt, tt, kcb)
                        tiles.append(xt)
                    return tiles

                ropeA_sb = wp.tile([P, S], BF, name="ropeA_sb")
                nc.gpsimd.dma_start(ropeA_sb[:], ropeA_d[:])
                ropeB_sb = wp.tile([P, S], BF, name="ropeB_sb")
                nc.gpsimd.dma_start(ropeB_sb[:], ropeB_d[:])
                ones_sb = const.tile([P, 1], BF, name="ones_sb")
                nc.gpsimd.dma_start(ones_sb[:], ones_d[:])
                tri_sb = const.tile([P, P], BF, name="tri_sb")
                nc.gpsimd.dma_start(tri_sb[:], tri_d[:])
                mask_sb = []
                for i in range(n_masks):
                    m = const.tile([P, 512], BF, name=f"mask{i}")
                    nc.gpsimd.dma_start(m[:], masks_d[i * P:(i + 1) * P, :])
                    mask_sb.append(m)
                qkvp = ph1.enter_context(
                    tc.tile_pool(name="qkvp", bufs=8, space="PSUM"))
                stg = ph1.enter_context(tc.tile_pool(name="stg", bufs=4))

                for tt in range(N_TT):
                    s0 = (tt % QT_N) * 512
                    xts = xts_cur
                    if tt + 1 < N_TT:
                        xts_next = load_x(tt + 1)
                    pss = [qkvp.tile([P, 512], F32, name="qkvps")
                           for _ in range(6)]
                    for kcb in range(KC // KCB):
                        for kc in range(KCB):
                            for f in range(6):
                                nc.tensor.matmul(
                                    pss[f][:],
                                    w_sb[kcb][:, kc, f * P:(f + 1) * P],
                                    xts[kcb][:, kc, :],
                                    start=(kcb == 0 and kc == 0),
                                    stop=(kcb == KC // KCB - 1 and
                                          kc == KCB - 1))
                    f_order = (5, 0, 1, 2, 3, 4) if tt == N_TT - 1 \
                        else (0, 1, 2, 3, 4, 5)
                    for fi, f in enumerate(f_order):
                        raw = stg.tile([P, 512], BF, name="raw")
                        # alternate eviction engines so the bank-reuse wait
                        # at the next token tile is halved
                        (nc.scalar.copy if fi % 2 == 0 else
                         nc.vector.tensor_copy)(raw[:], pss[f][:])
                        if f < 5:
                            sw_sb = stg.tile([P, 512], BF, name="sw_sb")
                            nc.gpsimd.dma_start(sw_sb[0::2, :], raw[1::2, :])
                            nc.gpsimd.dma_start(sw_sb[1::2, :], raw[0::2, :])
                            r1 = stg.tile([P, 512], BF, name="r1")
                            nc.vector.tensor_mul(r1[:], raw[:],
                                                 ropeA_sb[:, s0:s0 + 512])
                            r2 = stg.tile([P, 512], BF, name="r2")
                            nc.vector.tensor_mul(r2[:], sw_sb[:],
                                                 ropeB_sb[:, s0:s0 + 512])
                            dst = (qT[f] if f < HPC else kT)
                            nc.vector.tensor_add(
                                dst[:, tt * 512:(tt + 1) * 512], r1[:], r2[:])
                        else:
                            for j in range(4):
                                nc.sync.dma_start_transpose(
                                    vv[tt * 4 + j][:],
                                    raw[:, j * P:(j + 1) * P])
                    if tt + 1 < N_TT:
                        xts_cur = xts_next

            # yf[m]: [128, cc, 512] token-sharded y rows for head-group m
            # (allocated after ph1 so its SBUF doesn't collide with ph1 pools)
            yf = [actsB.tile([P, NCORES, 512], BF, name=f"yf{m}")
                  for m in range(HPC)]

            # wo prefetch pool opened early so its DMAs can stream during
            # attention; quarter-ot granularity [128, 8, 512] (1 MiB each)
            wop = top.enter_context(tc.tile_pool(name="wop", bufs=11))
            wo_tiles = {}

            def wo_prefetch(ot, qr, eng=None):
                """load wo chunk for output tile ot, dc quarter qr (8 dc)."""
                t = wop.tile([P, KC // 4, 512], BF, name="wo_sb")
                e = eng or (nc.scalar if (ot + qr) % 2 else nc.sync)
                src = woT_d[:, ot * 512:(ot + 1) * 512].rearrange(
                    "(qr dc p) o -> qr p dc o", p=P, dc=KC // 4)[qr]
                e.dma_start(t[:], src)
                wo_tiles[(ot, qr)] = t

            for ot in (0, 1):
                for qr in range(4):
                    wo_prefetch(ot, qr, nc.sync)

            # ---------------- phase 2: attention + A2A -------------------
            with ExitStack() as ph2, nc.named_scope("ph2_attn"):
                sp = ph2.enter_context(
                    tc.tile_pool(name="sp", bufs=3, space="PSUM"))
                yp = ph2.enter_context(
                    tc.tile_pool(name="yp", bufs=3, space="PSUM"))
                dp = ph2.enter_context(
                    tc.tile_pool(name="dp", bufs=2, space="PSUM"))
                ep = ph2.enter_context(tc.tile_pool(name="ep", bufs=8))
                recd = ph2.enter_context(
                    tc.tile_pool(name="recd", bufs=6, space="DRAM"))
                ys = ph2.enter_context(tc.tile_pool(name="ys", bufs=3))
                rp = ph2.enter_context(tc.tile_pool(name="rp", bufs=3))

                def emit_norm(h, b, qt, yu_sb, b_sb):
                    y_sb = ys.tile([P, 512], BF, name="y_sb")
                    nc.vector.tensor_mul(y_sb[:], yu_sb[:], b_sb[:])
                    r = b * QT_N + qt
                    nc.sync.dma_start(
                        a2a_in[h][r * P:(r + 1) * P, :], y_sb[:])

                yf_pending = []

                def flush_yf():
                    # issued from the gpsimd queue: it already serializes on
                    # the collective, so the wait on the A2A-done semaphore
                    # cannot head-of-line-block the sync DMA queue (which
                    # carries the reciprocal-broadcast chain)
                    while yf_pending:
                        hh = yf_pending.pop(0)
                        nc.gpsimd.dma_start(
                            yf[hh][:],
                            a2a_out[hh][:].rearrange(
                                "(cc p) t -> p cc t", p=P))

                # heads processed in pairs: the two heads at one (b,qt,kt)
                # share the kT / vv / ones stationary operands, so after
                # LDWEIGHTS dedupe every second matmul skips the reload
                for hp in range(HPC // 2):
                    heads = (hp, hp + 2)
                    pending = []
                    flush_yf()
                    for b in range(B):
                        for qt in range(QT_N):
                            kts = [kt for kt in range(KT_N)
                                   if cls[qt][kt] != 'skip']
                            pairs = _denom_pairs(kts, cls, qt)
                            ps_y = {h: yp.tile([P, 512], F32, name="psy")
                                    for h in heads}
                            ps_d = {h: dp.tile([1, 512], F32, name="psd")
                                    for h in heads}
                            es = {h: [None] * len(kts) for h in heads}
                            offs = [0] * len(kts)

                            def do_tile(i):
                                """scores (both heads) + exp + mask."""
                                kt = kts[i]
                                c = cls[qt][kt]
                                off = c[1] if isinstance(c, tuple) else 0
                                offs[i] = off
                                w_q = 512 - off
                                q0 = b * S + qt * 512 + off
                                pss = {}
                                for h in heads:
                                    ps_s = sp.tile([P, 512], F32,
                                                   name="pss")
                                    nc.tensor.matmul(
                                        ps_s[:, :w_q],
                                        kT[:, b * S + kt * P:
                                           b * S + (kt + 1) * P],
                                        qT[h][:, bass.ds(q0, w_q)],
                                        start=True, stop=True)
                                    pss[h] = ps_s
                                for h in heads:
                                    e = ep.tile([P, 512], BF, name="e")
                                    nc.scalar.activation(
                                        e[:, :w_q], pss[h][:, :w_q], EXP,
                                        scale=SCALE)
                                    if isinstance(c, tuple):
                                        nc.vector.tensor_mul(
                                            e[:, :P], e[:, :P], tri_sb[:])
                                    elif c != 'free':
                                        nc.vector.tensor_mul(
                                            e[:, :512], e[:, :512],
                                            mask_sb[c][:])
                                    es[h][i] = e

                            def do_av(i, first, last):
                                kt = kts[i]
                                off = offs[i]
                                for h in heads:
                                    nc.tensor.matmul(
                                        ps_y[h][:, off:512],
                                        vv[b * KT_N + kt][:],
                                        es[h][i][:, :512 - off],
                                        start=first, stop=last,
                                        skip_group_check=True)

                            def do_denom(rhs_f, off, first, last):
                                for h in heads:
                                    nc.tensor.matmul(
                                        ps_d[h][:, off:512], ones_sb[:],
                                        rhs_f(h)[:, :512 - off],
                                        start=first, stop=last,
                                        skip_group_check=True)

                            n_done = 0
                            n_total = len(kts)
                            first_d = True
                            for gi, grp in enumerate(pairs):
                                last_grp = (gi == len(pairs) - 1)
                                if grp[0] == 'single':
                                    i = grp[1]
                                    do_tile(i)
                                    do_av(i, n_done == 0,
                                          n_done + 1 == n_total)
                                    n_done += 1
                                    do_denom(lambda h: es[h][i], offs[i],
                                             first_d, last_grp)
                                else:
                                    ia, ib = grp[1], grp[2]
                                    do_tile(ia)
                                    do_tile(ib)
                                    do_av(ia, n_done == 0, False)
                                    do_av(ib, False,
                                          n_done + 2 == n_total)
                                    n_done += 2
                                    if grp[0] == 'pair':
                                        ss = {}
                                        for h in heads:
                                            s = ep.tile([P, 512], BF,
                                                        name="es")
                                            nc.vector.tensor_add(
                                                s[:], es[h][ia][:],
                                                es[h][ib][:])
                                            ss[h] = s
                                        do_denom(lambda h: ss[h], 0,
                                                 first_d, last_grp)
                                    else:
                                        # diag pair: fold the narrower
                                        # tile into the wider one
                                        oa, ob = offs[ia], offs[ib]
                                        w_b = 512 - ob
                                        for h in heads:
                                            nc.vector.tensor_add(
                                                es[h][ia][:, ob - oa:
                                                          ob - oa + w_b],
                                                es[h][ia][:, ob - oa:
                                                          ob - oa + w_b],
                                                es[h][ib][:, :w_b])
                                        do_denom(lambda h: es[h][ia], oa,
                                                 first_d, last_grp)
                                first_d = False
                            while len(pending) > 1:
                                emit_norm(*pending.pop(0))
                            for h in heads:
                                yu_sb = ys.tile([P, 512], F32,
                                                name="yu_sb")
                                nc.vector.tensor_copy(yu_sb[:],
                                                      ps_y[h][:])
                                rec = rp.tile([1, 512], F32, name="rec")
                                nc.vector.reciprocal_approx_fast(
                                    rec[:], ps_d[h][:])
                                rec_dram = recd.tile([1, 512], F32,
                                                     name="rec_dram")
                                nc.sync.dma_start(rec_dram[:], rec[:])
                                b_sb = ys.tile([P, 512], F32, name="b_sb")
                                nc.sync.dma_start(
                                    b_sb[:],
                                    rec_dram[:].partition_broadcast(P))
                                pending.append((h, b, qt, yu_sb, b_sb))
                    for pn in pending:
                        emit_norm(*pn)
                    pending = []
                    for h in heads:
                        nc.gpsimd.collective_compute(
                            "AllToAll", mybir.AluOpType.bypass,
                            replica_groups=rg,
                            ins=[a2a_in[h].opt()], outs=[a2a_out[h].opt()])
                        yf_pending.append(h)
                flush_yf()

            # ---------------- phase 3: out projection --------------------
            # ot pairs with dc-outer / ts-inner ordering: the (ot, ot+1)
            # matmuls at one (dc, ts) share the yf stationary chunk (LDW
            # dedupe), wo quarters free progressively as dc advances, and
            # the 8 psum accumulators use all banks
            with ExitStack() as ph3, nc.named_scope("ph3_outp"):
                opp = ph3.enter_context(
                    tc.tile_pool(name="opp", bufs=8, space="PSUM"))
                osb = ph3.enter_context(tc.tile_pool(name="osb", bufs=2))

                def mm(pso, ot, ts, dc, start, stop):
                    nc.tensor.matmul(
                        pso[:], yf[dc // NCORES][:, dc % NCORES,
                                                 ts * P:(ts + 1) * P],
                        wo_tiles[(ot, dc // 8)][:, dc % 8, :],
                        start=start, stop=stop, skip_group_check=True)

                prefetch_q = [(ot, qr) for ot in range(2, 8)
                              for qr in range(4)]

                def evict(ot, psos):
                    ob = osb.tile([P, 4, 512], F32, name="ob")
                    for ts in range(4):
                        (nc.scalar.copy if ts % 2 == 0 else
                         nc.vector.tensor_copy)(ob[:, ts, :], psos[ts][:])
                    nc.sync.dma_start(
                        out_d[:, ot * 512:(ot + 1) * 512].rearrange(
                            "(ts p) o -> p ts o", p=P),
                        ob[:])
                    for qr in range(4):
                        del wo_tiles[(ot, qr)]
                    for _ in range(4):
                        if prefetch_q:
                            wo_prefetch(*prefetch_q.pop(0))

                def run_pair(ota, otb, dc_ranges):
                    psos = {ot: [opp.tile([P, 512], F32, name="pso")
                                 for _ in range(4)] for ot in (ota, otb)}
                    for dcs in dc_ranges:
                        for dc in dcs:
                            for ts in range(4):
                                for ot in (ota, otb):
                                    mm(psos[ot][ts], ot, ts, dc,
                                       dc == 0, dc == KC - 1)
                    evict(ota, psos[ota])
                    evict(otb, psos[otb])

                # first pair: dc 0..23 (m<=2) before dc 24..31 so the last
                # AllToAlls hide under ~45us of independent matmuls
                run_pair(0, 1, (list(range(0, 8)) + list(range(16, 24)),
                                list(range(8, 16)) + list(range(24, KC))))
                run_pair(2, 3, (range(KC),))
                run_pair(4, 5, (range(KC),))
                # last pair: ts-outer so accumulators finish staggered and
                # the final evictions overlap the remaining matmuls
                psos = {ot: [opp.tile([P, 512], F32, name="pso")
                             for _ in range(4)] for ot in (6, 7)}
                obs = {ot: osb.tile([P, 4, 512], F32, name="ob")
                       for ot in (6, 7)}
                for ts in range(4):
                    for dc in range(KC):
                        for ot in (6, 7):
                            mm(psos[ot][ts], ot, ts, dc,
                               dc == 0, dc == KC - 1)
                    for ot in (6, 7):
                        (nc.scalar.copy if ot == 6 else
                         nc.vector.tensor_copy)(obs[ot][:, ts, :],
                                                psos[ot][ts][:])
                for ot in (6, 7):
                    nc.sync.dma_start(
                        out_d[:, ot * 512:(ot + 1) * 512].rearrange(
                            "(ts p) o -> p ts o", p=P), obs[ot][:])

    nc.compile()
    _dedupe_ldweights(nc)
    return nc


# ---------------------------------------------------------------------------
# public entry
# ---------------------------------------------------------------------------

_CACHE = {}


def _execute(x, freqs_cis, mask_cache, input_pos, wqkv, wo,
             trace=False, debug=False):
    _install_profile_shim()
    prep = _prep(x, freqs_cis, mask_cache, wqkv, wo)
    cls = prep['cls']
    n_masks = 0 if prep['masks'] is None else prep['masks'].shape[0] // P
    key = (str(cls), n_masks, debug)
    if key not in _CACHE:
        _CACHE[key] = _build(cls, n_masks, debug=debug)
    nc = _CACHE[key]

    in_maps = []
    for c in range(NCORES):
        m = dict(xT=prep['xT'], wT=prep['wTs'][c], woT=prep['woT'],
                 ropeA=prep['ropeA'], ropeB=prep['ropeB'],
                 ones=prep['ones'], tri=prep['tri'])
        if n_masks:
            m['masks'] = prep['masks']
        in_maps.append(m)

    res = run_bass_kernel_spmd(nc, in_maps, core_ids=list(range(NCORES)),
                               trace=trace,
                               trace_cores=list(range(NCORES)) if trace
                               else None)
    out = np.zeros((B, S, DIM), dtype=np.float32)
    for c in range(NCORES):
        b, j = c // QT_N, c % QT_N
        out[b, j * 512:(j + 1) * 512] = res.results[c]['out']
    return out, res


def kernel(x, freqs_cis, mask_cache, input_pos, wqkv, wo):
    out, _ = _execute(x, freqs_cis, mask_cache, input_pos, wqkv, wo)
    return out


# ---------------------------------------------------------------------------
# numpy simulation of the exact device pipeline (for validation)
# ---------------------------------------------------------------------------

def _simulate(x, freqs_cis, mask_cache, wqkv, wo, use_bf16=True):
    """Mirror the device computation in numpy.  Returns (out, debug_dict)."""
    def q_(a):  # quantize
        return a.astype(bf16).astype(np.float32) if use_bf16 else a

    prep = _prep(x, freqs_cis, mask_cache, wqkv, wo)
    cls = prep['cls']
    xT = prep['xT'].astype(np.float32)
    ropeA = np.concatenate([prep['ropeA'].astype(np.float32)] * B, axis=1)
    ropeB = np.concatenate([prep['ropeB'].astype(np.float32)] * B, axis=1)
    mask = np.asarray(mask_cache)[0, 0]

    dbg = {c: {} for c in range(NCORES)}
    a2a_ins = {m: [] for m in range(HPC)}  # m -> [core][8*128, 512]
    for c in range(NCORES):
        wT = prep['wTs'][c].astype(np.float32)
        qkvT = q_(wT.T @ xT)       # [768, TOK]  (psum f32, evict to bf16)
        sw = np.empty_like(qkvT[:5 * P])
        for f in range(5):
            blk = qkvT[f * P:(f + 1) * P]
            sw[f * P:(f + 1) * P] = q_(blk[[i ^ 1 for i in range(P)], :])
        roped = np.empty_like(qkvT[:5 * P])
        for f in range(5):
            blk = qkvT[f * P:(f + 1) * P]
            r1 = q_(blk * ropeA)
            r2 = q_(sw[f * P:(f + 1) * P] * ropeB)
            roped[f * P:(f + 1) * P] = q_(r1 + r2)
        qTs = [roped[h * P:(h + 1) * P] for h in range(HPC)]
        kTc = roped[4 * P:5 * P]
        vT = qkvT[5 * P:6 * P]     # [128 d, TOK], not roped
        for h in range(HPC):
            a2a_c = np.zeros((NCORES * P, 512), dtype=np.float32)
            for b in range(B):
                kTb = kTc[:, b * S:(b + 1) * S]
                vTb = vT[:, b * S:(b + 1) * S]
                qTb = qTs[h][:, b * S:(b + 1) * S]
                sT = kTb.T @ qTb               # [Sk, Sq] psum f32
                e = q_(np.exp(sT * SCALE))     # ACT exp -> bf16
                emask = e * mask.T             # mask multiply (exact 0/1)
                for qt in range(QT_N):
                    for kt in range(KT_N):
                        if cls[qt][kt] == 'skip':
                            emask[kt * P:(kt + 1) * P,
                                  qt * 512:(qt + 1) * 512] = 0
                # denominator via bf16 pair sums, accumulated in f32
                D = np.zeros(S, dtype=np.float32)
                for qt in range(QT_N):
                    kts = [kt for kt in range(KT_N)
                           if cls[qt][kt] != 'skip']
                    pairs = _denom_pairs(kts, cls, qt)
                    qs = slice(qt * 512, (qt + 1) * 512)
                    for grp in pairs:
                        if grp[0] == 'single':
                            kt = kts[grp[1]]
                            D[qs] += emask[kt * P:(kt + 1) * P, qs].sum(0)
                        else:
                            ka, kb = kts[grp[1]], kts[grp[2]]
                            ps = q_(emask[ka * P:(ka + 1) * P, qs] +
                                    emask[kb * P:(kb + 1) * P, qs])
                            D[qs] += ps.sum(0)
                rec = 1.0 / D
                yTu = vTb @ emask
                y = q_(yTu * rec[None, :])
                for qt in range(QT_N):
                    r = b * QT_N + qt
                    a2a_c[r * P:(r + 1) * P] = y[:, qt * 512:(qt + 1) * 512]
            a2a_ins[h].append(a2a_c)

    # route the A2As:  out shard j on rank c = rank j's input shard c
    out_full = np.zeros((B, S, DIM), dtype=np.float32)
    woT = prep['woT'].astype(np.float32)
    for c in range(NCORES):
        yfull = np.zeros((DIM, 512), dtype=np.float32)
        for m in range(HPC):
            for j in range(NCORES):
                dbi = m * NCORES + j
                yfull[dbi * P:(dbi + 1) * P] = \
                    a2a_ins[m][j][c * P:(c + 1) * P]
        o = yfull.T @ woT          # [512 tok, DIM] psum f32
        b, jj = c // QT_N, c % QT_N
        out_full[b, jj * 512:(jj + 1) * 512] = o
    return out_full, dbg


# revision 27
# speedup vs baseline: 1.0196x; 1.0196x over previous
"""Distributed Trainium2 kernel for nn_Attention_37958920962105.

GQA attention layer (DIM=4096, 32 q heads, 8 kv heads, head_dim=128,
B=2, S=2048) sharded tensor-parallel over GQA groups across 8 cores:
core c owns q heads 4c..4c+3 and kv head c.  Per core:
  1. QKV projection (transposed layouts) + RoPE (pair-swap via strided
     SBUF->SBUF DMA) + v transpose (XBAR DMA transpose)
  2. attention (scores -> exp -> paired-denominator matmuls -> AV)
  3. AllToAll (one per local head) to token-shard y
  4. out projection on this core's 512-token chunk (first two output
     tiles run their m<=2 chunks first so the last AllToAll hides)
Output chunks are reassembled on the host.
"""

import math
import sys
import types
from contextlib import ExitStack

import numpy as np
import ml_dtypes

import concourse.bass as bass
import concourse.mybir as mybir
import concourse.tile as tile
from concourse import bacc
from concourse.bass_utils import run_bass_kernel_spmd

BF = mybir.dt.bfloat16
F32 = mybir.dt.float32
bf16 = ml_dtypes.bfloat16

P = 128
DIM = 4096
N_HEAD = 32
N_KV = 8
HD = 128
B = 2
S = 2048
TOK = B * S          # 4096
NCORES = 8
HPC = N_HEAD // N_KV  # 4 q heads per core
FQKV = (HPC + 2) * HD  # 768 qkv rows per core
KC = DIM // P        # 32 contraction chunks
KCB = 8              # kc chunks per batched DMA
N_TT = TOK // 512    # 8 token tiles of 512
QT_N = S // 512      # 4 q tiles per batch
KT_N = S // P        # 16 k tiles per batch
SCALE = 1.0 / math.sqrt(HD)


def _install_profile_shim():
    if 'antenv.axon_hooks' in sys.modules:
        return
    try:
        from trn_agent_boot.trn_boot import _ntff_profile_via_ctypes
        hook = _ntff_profile_via_ctypes('/opt/axon/libaxon_pjrt.so')
    except Exception:
        hook = None
    mod = types.ModuleType('antenv.axon_hooks')
    mod._hook = hook
    mod.get_axon_ntff_profile_hook = lambda: mod._hook
    mod.set_axon_ntff_profile_hook = lambda h: setattr(mod, '_hook', h)
    sys.modules['antenv.axon_hooks'] = mod
    try:
        import antenv
        antenv.axon_hooks = mod
    except ImportError:
        pass


# ---------------------------------------------------------------------------
# host-side prep
# ---------------------------------------------------------------------------

def _classify_mask(mask):
    """mask: [S(q), S(k)] bool.  Returns (cls, mask_tiles) where
    cls[qt][kt] in {'skip', 'free', int mask-tile-index}; mask tiles are
    transposed [128 k, 512 q] bf16."""
    cls = [[None] * KT_N for _ in range(QT_N)]
    tiles = []
    seen = {}
    qi = np.arange(512)[:, None]
    ki = np.arange(P)[None, :]
    for qt in range(QT_N):
        for kt in range(KT_N):
            blk = mask[qt * 512:(qt + 1) * 512, kt * P:(kt + 1) * P]
            if not blk.any():
                cls[qt][kt] = 'skip'
                continue
            if blk.all():
                cls[qt][kt] = 'free'
                continue
            off = kt * P - qt * 512
            if 0 <= off < 512 and np.array_equal(blk, (off + ki) <= qi):
                # causal diagonal tile: valid only for q >= off, and within
                # the first 128 valid q columns it is the k<=q' triangle
                cls[qt][kt] = ('diag', off)
                continue
            key = blk.tobytes()
            if key not in seen:
                seen[key] = len(tiles)
                tiles.append(np.ascontiguousarray(blk.T).astype(bf16))
            cls[qt][kt] = seen[key]
    return cls, tiles


def _prep(x, freqs_cis, mask_cache, wqkv, wo):
    x = np.asarray(x, dtype=np.float32)
    freqs_cis = np.asarray(freqs_cis, dtype=np.float32)
    wqkv = np.asarray(wqkv, dtype=np.float32)
    wo = np.asarray(wo, dtype=np.float32)
    mask = np.asarray(mask_cache)[0, 0]

    xT = np.ascontiguousarray(x.reshape(TOK, DIM).T).astype(bf16)

    wTs = []
    for c in range(NCORES):
        w_c = np.concatenate([
            wqkv[HPC * HD * c: HPC * HD * (c + 1)],          # 4 q heads
            wqkv[N_HEAD * HD + HD * c: N_HEAD * HD + HD * (c + 1)],   # k head
            wqkv[(N_HEAD + N_KV) * HD + HD * c:
                 (N_HEAD + N_KV) * HD + HD * (c + 1)],       # v head
        ], axis=0)                                           # [768, DIM]
        wTs.append(np.ascontiguousarray(w_c.T).astype(bf16))  # [DIM, 768]

    # wo permuted so row-block dbi = m*8 + cc holds global head 4*cc + m
    woT = np.ascontiguousarray(wo.T)                 # [d, o]
    woT_h = woT.reshape(N_HEAD, HD, DIM)
    perm = [4 * (dbi % NCORES) + dbi // NCORES for dbi in range(N_HEAD)]
    woT_perm = np.ascontiguousarray(woT_h[perm].reshape(DIM, DIM)).astype(bf16)

    f0 = freqs_cis[:, :, 0].T                        # [64, S]
    f1 = freqs_cis[:, :, 1].T
    ropeA = np.repeat(f0, 2, axis=0).astype(bf16)    # [128, S]
    ropeB = np.empty((HD, S), dtype=np.float32)
    ropeB[0::2] = -f1
    ropeB[1::2] = f1
    ropeB = ropeB.astype(bf16)

    ones_col = np.ones((P, 1), dtype=bf16)

    tri = (np.arange(P)[:, None] <= np.arange(P)[None, :]).astype(bf16)
    cls, mask_tiles = _classify_mask(mask)
    masks = (np.concatenate([t for t in mask_tiles], axis=0)
             if mask_tiles else None)                # [n*128, 512] bf16

    return dict(xT=xT, wTs=wTs, woT=woT_perm, ropeA=ropeA, ropeB=ropeB,
                ones=ones_col, tri=tri, cls=cls, masks=masks)


# ---------------------------------------------------------------------------
# device kernel builder
# ---------------------------------------------------------------------------

def _denom_pairs(kts, cls, qt):
    """Group the kt tiles of one (qt) into pairs for the denominator
    matmuls.  Returns a list of ('pair', i, j) / ('single', i) entries
    over indices into kts.  Free tiles are paired in order; the 4 diag
    tiles are paired (w512,w384) and (w256,w128)."""
    free = [i for i, kt in enumerate(kts) if cls[qt][kt] == 'free']
    diag = [i for i, kt in enumerate(kts) if isinstance(cls[qt][kt], tuple)]
    out = []
    for a in range(0, len(free) - 1, 2):
        out.append(('pair', free[a], free[a + 1]))
    if len(free) % 2:
        out.append(('single', free[-1]))
    # diag tiles sorted by offset ascending (w descending)
    diag.sort(key=lambda i: cls[qt][kts[i]][1])
    for a in range(0, len(diag) - 1, 2):
        out.append(('dpair', diag[a], diag[a + 1]))
    if len(diag) % 2:
        out.append(('single', diag[-1]))
    return out


def _dedupe_ldweights(nc):
    """Remove InstLdweights whose weights AP matches the previous PE
    weight load: the PE array already holds those weights, and skipping
    the reload removes the ~128-cycle drain-before-weight-swap stall
    between back-to-back matmuls.  Only drops loads with no semaphore
    waits/updates.  Safe here because every stationary operand in this
    kernel lives in a non-rotating SBUF tile (w_sb, kT, vv, ones, yf)."""
    n_removed = 0
    for fn in nc.m.functions:
        for bb in fn.blocks:
            lst = bb.instructions
            out = []
            prev_sig = None
            changed = False
            for inst in lst:
                nm = type(inst).__name__
                if nm == 'InstLdweights':
                    sig = (str(inst.ins[0]), str(inst.perf_mode),
                           str(inst.is_transpose), str(inst.tile_position))
                    if sig == prev_sig and inst.sync_info is None:
                        n_removed += 1
                        changed = True
                        continue
                    prev_sig = sig
                elif nm == 'InstMatmult':
                    if inst.is_transpose:
                        prev_sig = None
                elif nm in ('InstDrain', 'InstISA', 'InstCall',
                            'InstUnconditionalBranch'):
                    prev_sig = None
                out.append(inst)
            if changed:
                bb.instructions = out
    return n_removed


def _build(cls, n_masks, debug=False):
    nc = bacc.Bacc("TRN2", target_bir_lowering=False, debug=False,
                   num_devices=NCORES)
    xT_d = nc.dram_tensor("xT", [DIM, TOK], BF, kind="ExternalInput")
    wT_d = nc.dram_tensor("wT", [DIM, FQKV], BF, kind="ExternalInput")
    woT_d = nc.dram_tensor("woT", [DIM, DIM], BF, kind="ExternalInput")
    ropeA_d = nc.dram_tensor("ropeA", [P, S], BF, kind="ExternalInput")
    ropeB_d = nc.dram_tensor("ropeB", [P, S], BF, kind="ExternalInput")
    ones_d = nc.dram_tensor("ones", [P, 1], BF, kind="ExternalInput")
    tri_d = nc.dram_tensor("tri", [P, P], BF, kind="ExternalInput")
    masks_d = (nc.dram_tensor("masks", [n_masks * P, 512], BF,
                              kind="ExternalInput") if n_masks else None)
    out_d = nc.dram_tensor("out", [512, DIM], F32, kind="ExternalOutput")

    EXP = mybir.ActivationFunctionType.Exp
    rg = [list(range(NCORES))]

    with tile.TileContext(nc) as tc:
        with ExitStack() as top:
            const = top.enter_context(tc.tile_pool(name="const", bufs=1))
            actsA = top.enter_context(tc.tile_pool(name="actsA", bufs=1))
            actsB = top.enter_context(tc.tile_pool(name="actsB", bufs=1))
            dramp = top.enter_context(tc.tile_pool(name="dramp", bufs=1,
                                                   space="DRAM"))

            qT = [actsA.tile([P, TOK], BF, name=f"qT{h}") for h in range(HPC)]
            kT = actsA.tile([P, TOK], BF, name="kT")
            vv = [actsA.tile([P, P], BF, name=f"v{i}") for i in range(TOK // P)]

            a2a_in = [dramp.tile([NCORES * P, 512], BF, name=f"a2ai{m}")
                      for m in range(HPC)]
            a2a_out = [dramp.tile([NCORES * P, 512], BF, name=f"a2ao{m}")
                       for m in range(HPC)]

            # ---------------- phase 1: QKV + rope + v transpose ----------
            with ExitStack() as ph1, nc.named_scope("ph1_qkv"):
                wp = ph1.enter_context(tc.tile_pool(name="wp", bufs=1))
                xp = ph1.enter_context(tc.tile_pool(name="xp", bufs=6))
                w_src = wT_d[:].rearrange("(kc p) f -> kc p f", p=P)
                x_src = xT_d[:].rearrange("(kc p) t -> kc p t", p=P)

                def load_x_batch(xt, tt, kcb):
                    (nc.scalar if kcb % 2 else nc.sync).dma_start(
                        xt[:], xT_d[:, tt * 512:(tt + 1) * 512].rearrange(
                            "(kcb kc p) t -> kcb p kc t", p=P, kc=KCB)[kcb])

                # fine-grained interleaved first chunks so the very first
                # matmul can start after ~400 KiB instead of 2.5 MiB
                w0 = wp.tile([P, KCB, FQKV], BF, name="w0")
                x0 = xp.tile([P, KCB, 512], BF, name="xt")
                for kc in range(KCB):
                    (nc.sync if kc % 2 == 0 else nc.scalar).dma_start(
                        w0[:, kc, :], w_src[kc])
                    (nc.scalar if kc % 2 == 0 else nc.sync).dma_start(
                        x0[:, kc, :], x_src[kc, :, 0:512])
                w_sb = [w0]
                xts_cur = [x0]
                for kcb in range(1, KC // KCB):
                    w = wp.tile([P, KCB, FQKV], BF, name=f"w{kcb}")
                    (nc.sync if kcb % 2 else nc.scalar).dma_start(
                        w[:], wT_d[:].rearrange(
                            "(kcb kc p) f -> kcb p kc f", p=P, kc=KCB
                        )[kcb])
                    w_sb.append(w)
                    xt = xp.tile([P, KCB, 512], BF, name="xt")
                    load_x_batch(xt, 0, kcb)
                    xts_cur.append(xt)

                def load_x(tt):
                    tiles = []
                    for kcb in range(KC // KCB):
                        xt = xp.tile([P, KCB, 512], BF, name="xt")
                        load_x_batch(xt, tt, kcb)
                        tiles.append(xt)
                    return tiles

                ropeA_sb = wp.tile([P, S], BF, name="ropeA_sb")
                nc.sync.dma_start(ropeA_sb[:], ropeA_d[:])
                ropeB_sb = wp.tile([P, S], BF, name="ropeB_sb")
                nc.sync.dma_start(ropeB_sb[:], ropeB_d[:])
                ones_sb = const.tile([P, 1], BF, name="ones_sb")
                nc.sync.dma_start(ones_sb[:], ones_d[:])
                tri_sb = const.tile([P, P], BF, name="tri_sb")
                nc.sync.dma_start(tri_sb[:], tri_d[:])
                mask_sb = []
                for i in range(n_masks):
                    m = const.tile([P, 512], BF, name=f"mask{i}")
                    nc.sync.dma_start(m[:], masks_d[i * P:(i + 1) * P, :])
                    mask_sb.append(m)
                qkvp = ph1.enter_context(
                    tc.tile_pool(name="qkvp", bufs=8, space="PSUM"))
                stg = ph1.enter_context(tc.tile_pool(name="stg", bufs=4))

                for tt in range(N_TT):
                    s0 = (tt % QT_N) * 512
                    xts = xts_cur
                    if tt + 1 < N_TT:
                        xts_next = load_x(tt + 1)
                    pss = [qkvp.tile([P, 512], F32, name="qkvps")
                           for _ in range(6)]
                    for kcb in range(KC // KCB):
                        for kc in range(KCB):
                            for f in range(6):
                                nc.tensor.matmul(
                                    pss[f][:],
                                    w_sb[kcb][:, kc, f * P:(f + 1) * P],
                                    xts[kcb][:, kc, :],
                                    start=(kcb == 0 and kc == 0),
                                    stop=(kcb == KC // KCB - 1 and
                                          kc == KCB - 1))
                    f_order = (5, 0, 1, 2, 3, 4) if tt == N_TT - 1 \
                        else (0, 1, 2, 3, 4, 5)
                    for fi, f in enumerate(f_order):
                        raw = stg.tile([P, 512], BF, name="raw")
                        # alternate eviction engines so the bank-reuse wait
                        # at the next token tile is halved
                        (nc.scalar.copy if fi % 2 == 0 else
                         nc.vector.tensor_copy)(raw[:], pss[f][:])
                        if f < 5:
                            sw_sb = stg.tile([P, 512], BF, name="sw_sb")
                            nc.sync.dma_start(sw_sb[0::2, :], raw[1::2, :])
                            nc.scalar.dma_start(sw_sb[1::2, :], raw[0::2, :])
                            r1 = stg.tile([P, 512], BF, name="r1")
                            nc.vector.tensor_mul(r1[:], raw[:],
                                                 ropeA_sb[:, s0:s0 + 512])
                            r2 = stg.tile([P, 512], BF, name="r2")
                            nc.vector.tensor_mul(r2[:], sw_sb[:],
                                                 ropeB_sb[:, s0:s0 + 512])
                            dst = (qT[f] if f < HPC else kT)
                            nc.vector.tensor_add(
                                dst[:, tt * 512:(tt + 1) * 512], r1[:], r2[:])
                        else:
                            for j in range(4):
                                nc.sync.dma_start_transpose(
                                    vv[tt * 4 + j][:],
                                    raw[:, j * P:(j + 1) * P])
                    if tt + 1 < N_TT:
                        xts_cur = xts_next

            # yf[m]: [128, cc, 512] token-sharded y rows for head-group m
            # (allocated after ph1 so its SBUF doesn't collide with ph1 pools)
            yf = [actsB.tile([P, NCORES, 512], BF, name=f"yf{m}")
                  for m in range(HPC)]

            # wo prefetch pool opened early so its DMAs can stream during
            # attention; quarter-ot granularity [128, 8, 512] (1 MiB each)
            wop = top.enter_context(tc.tile_pool(name="wop", bufs=11))
            wo_tiles = {}

            def wo_prefetch(ot, qr, eng=None):
                """load wo chunk for output tile ot, dc quarter qr (8 dc)."""
                t = wop.tile([P, KC // 4, 512], BF, name="wo_sb")
                e = eng or (nc.scalar if (ot + qr) % 2 else nc.sync)
                src = woT_d[:, ot * 512:(ot + 1) * 512].rearrange(
                    "(qr dc p) o -> qr p dc o", p=P, dc=KC // 4)[qr]
                e.dma_start(t[:], src)
                wo_tiles[(ot, qr)] = t

            for ot in (0, 1):
                for qr in range(4):
                    wo_prefetch(ot, qr, nc.sync)

            # ---------------- phase 2: attention + A2A -------------------
            with ExitStack() as ph2, nc.named_scope("ph2_attn"):
                sp = ph2.enter_context(
                    tc.tile_pool(name="sp", bufs=3, space="PSUM"))
                yp = ph2.enter_context(
                    tc.tile_pool(name="yp", bufs=3, space="PSUM"))
                dp = ph2.enter_context(
                    tc.tile_pool(name="dp", bufs=2, space="PSUM"))
                ep = ph2.enter_context(tc.tile_pool(name="ep", bufs=8))
                recd = ph2.enter_context(
                    tc.tile_pool(name="recd", bufs=6, space="DRAM"))
                ys = ph2.enter_context(tc.tile_pool(name="ys", bufs=3))
                rp = ph2.enter_context(tc.tile_pool(name="rp", bufs=3))

                def emit_norm(h, b, qt, yu_sb, b_sb):
                    y_sb = ys.tile([P, 512], BF, name="y_sb")
                    nc.vector.tensor_mul(y_sb[:], yu_sb[:], b_sb[:])
                    r = b * QT_N + qt
                    nc.sync.dma_start(
                        a2a_in[h][r * P:(r + 1) * P, :], y_sb[:])

                yf_pending = []

                def flush_yf():
                    # issued from the gpsimd queue: it already serializes on
                    # the collective, so the wait on the A2A-done semaphore
                    # cannot head-of-line-block the sync DMA queue (which
                    # carries the reciprocal-broadcast chain)
                    while yf_pending:
                        hh = yf_pending.pop(0)
                        nc.gpsimd.dma_start(
                            yf[hh][:],
                            a2a_out[hh][:].rearrange(
                                "(cc p) t -> p cc t", p=P))

                # heads processed in pairs: the two heads at one (b,qt,kt)
                # share the kT / vv / ones stationary operands, so after
                # LDWEIGHTS dedupe every second matmul skips the reload
                for heads in ((0, 2), (1,), (3,)):
                    pending = []
                    flush_yf()
                    for b in range(B):
                        for qt in range(QT_N):
                            kts = [kt for kt in range(KT_N)
                                   if cls[qt][kt] != 'skip']
                            pairs = _denom_pairs(kts, cls, qt)
                            ps_y = {h: yp.tile([P, 512], F32, name="psy")
                                    for h in heads}
                            ps_d = {h: dp.tile([1, 512], F32, name="psd")
                                    for h in heads}
                            es = {h: [None] * len(kts) for h in heads}
                            offs = [0] * len(kts)

                            def do_tile(i):
                                """scores (both heads) + exp + mask."""
                                kt = kts[i]
                                c = cls[qt][kt]
                                off = c[1] if isinstance(c, tuple) else 0
                                offs[i] = off
                                w_q = 512 - off
                                q0 = b * S + qt * 512 + off
                                pss = {}
                                for h in heads:
                                    ps_s = sp.tile([P, 512], F32,
                                                   name="pss")
                                    nc.tensor.matmul(
                                        ps_s[:, :w_q],
                                        kT[:, b * S + kt * P:
                                           b * S + (kt + 1) * P],
                                        qT[h][:, bass.ds(q0, w_q)],
                                        start=True, stop=True)
                                    pss[h] = ps_s
                                for h in heads:
                                    e = ep.tile([P, 512], BF, name="e")
                                    nc.scalar.activation(
                                        e[:, :w_q], pss[h][:, :w_q], EXP,
                                        scale=SCALE)
                                    if isinstance(c, tuple):
                                        nc.vector.tensor_mul(
                                            e[:, :P], e[:, :P], tri_sb[:])
                                    elif c != 'free':
                                        nc.vector.tensor_mul(
                                            e[:, :512], e[:, :512],
                                            mask_sb[c][:])
                                    es[h][i] = e

                            def do_av(i, first, last):
                                kt = kts[i]
                                off = offs[i]
                                for h in heads:
                                    nc.tensor.matmul(
                                        ps_y[h][:, off:512],
                                        vv[b * KT_N + kt][:],
                                        es[h][i][:, :512 - off],
                                        start=first, stop=last,
                                        skip_group_check=True)

                            def do_denom(rhs_f, off, first, last):
                                for h in heads:
                                    nc.tensor.matmul(
                                        ps_d[h][:, off:512], ones_sb[:],
                                        rhs_f(h)[:, :512 - off],
                                        start=first, stop=last,
                                        skip_group_check=True)

                            n_done = 0
                            n_total = len(kts)
                            first_d = True
                            for gi, grp in enumerate(pairs):
                                last_grp = (gi == len(pairs) - 1)
                                if grp[0] == 'single':
                                    i = grp[1]
                                    do_tile(i)
                                    do_av(i, n_done == 0,
                                          n_done + 1 == n_total)
                                    n_done += 1
                                    do_denom(lambda h: es[h][i], offs[i],
                                             first_d, last_grp)
                                else:
                                    ia, ib = grp[1], grp[2]
                                    do_tile(ia)
                                    do_tile(ib)
                                    do_av(ia, n_done == 0, False)
                                    do_av(ib, False,
                                          n_done + 2 == n_total)
                                    n_done += 2
                                    if grp[0] == 'pair':
                                        ss = {}
                                        for h in heads:
                                            s = ep.tile([P, 512], BF,
                                                        name="es")
                                            nc.vector.tensor_add(
                                                s[:], es[h][ia][:],
                                                es[h][ib][:])
                                            ss[h] = s
                                        do_denom(lambda h: ss[h], 0,
                                                 first_d, last_grp)
                                    else:
                                        # diag pair: fold the narrower
                                        # tile into the wider one
                                        oa, ob = offs[ia], offs[ib]
                                        w_b = 512 - ob
                                        for h in heads:
                                            nc.vector.tensor_add(
                                                es[h][ia][:, ob - oa:
                                                          ob - oa + w_b],
                                                es[h][ia][:, ob - oa:
                                                          ob - oa + w_b],
                                                es[h][ib][:, :w_b])
                                        do_denom(lambda h: es[h][ia], oa,
                                                 first_d, last_grp)
                                first_d = False
                            while len(pending) > 1:
                                emit_norm(*pending.pop(0))
                            for h in heads:
                                yu_sb = ys.tile([P, 512], F32,
                                                name="yu_sb")
                                nc.vector.tensor_copy(yu_sb[:],
                                                      ps_y[h][:])
                                rec = rp.tile([1, 512], F32, name="rec")
                                nc.vector.reciprocal_approx_fast(
                                    rec[:], ps_d[h][:])
                                rec_dram = recd.tile([1, 512], F32,
                                                     name="rec_dram")
                                nc.sync.dma_start(rec_dram[:], rec[:])
                                b_sb = ys.tile([P, 512], F32, name="b_sb")
                                nc.sync.dma_start(
                                    b_sb[:],
                                    rec_dram[:].partition_broadcast(P))
                                pending.append((h, b, qt, yu_sb, b_sb))
                    for pn in pending:
                        emit_norm(*pn)
                    pending = []
                    for h in heads:
                        nc.gpsimd.collective_compute(
                            "AllToAll", mybir.AluOpType.bypass,
                            replica_groups=rg,
                            ins=[a2a_in[h].opt()], outs=[a2a_out[h].opt()])
                        yf_pending.append(h)
                flush_yf()

            # ---------------- phase 3: out projection --------------------
            # ot pairs with dc-outer / ts-inner ordering: the (ot, ot+1)
            # matmuls at one (dc, ts) share the yf stationary chunk (LDW
            # dedupe), wo quarters free progressively as dc advances, and
            # the 8 psum accumulators use all banks
            with ExitStack() as ph3, nc.named_scope("ph3_outp"):
                opp = ph3.enter_context(
                    tc.tile_pool(name="opp", bufs=8, space="PSUM"))
                osb = ph3.enter_context(tc.tile_pool(name="osb", bufs=2))

                def mm(pso, ot, ts, dc, start, stop):
                    nc.tensor.matmul(
                        pso[:], yf[dc // NCORES][:, dc % NCORES,
                                                 ts * P:(ts + 1) * P],
                        wo_tiles[(ot, dc // 8)][:, dc % 8, :],
                        start=start, stop=stop, skip_group_check=True)

                prefetch_q = [(ot, qr) for ot in range(2, 8)
                              for qr in range(4)]

                def evict(ot, psos):
                    ob = osb.tile([P, 4, 512], F32, name="ob")
                    for ts in range(4):
                        (nc.scalar.copy if ts % 2 == 0 else
                         nc.vector.tensor_copy)(ob[:, ts, :], psos[ts][:])
                    nc.sync.dma_start(
                        out_d[:, ot * 512:(ot + 1) * 512].rearrange(
                            "(ts p) o -> p ts o", p=P),
                        ob[:])
                    for qr in range(4):
                        del wo_tiles[(ot, qr)]
                    for _ in range(4):
                        if prefetch_q:
                            wo_prefetch(*prefetch_q.pop(0))

                def run_pair(ota, otb, dc_ranges):
                    psos = {ot: [opp.tile([P, 512], F32, name="pso")
                                 for _ in range(4)] for ot in (ota, otb)}
                    for dcs in dc_ranges:
                        for dc in dcs:
                            for ts in range(4):
                                for ot in (ota, otb):
                                    mm(psos[ot][ts], ot, ts, dc,
                                       dc == 0, dc == KC - 1)
                    evict(ota, psos[ota])
                    evict(otb, psos[otb])

                # first pair: dc 0..23 (m<=2) before dc 24..31 so the last
                # AllToAlls hide under ~45us of independent matmuls
                run_pair(0, 1, (range(24), range(24, KC)))
                run_pair(2, 3, (range(KC),))
                run_pair(4, 5, (range(KC),))
                # last pair: ts-outer so accumulators finish staggered and
                # the final evictions overlap the remaining matmuls
                psos = {ot: [opp.tile([P, 512], F32, name="pso")
                             for _ in range(4)] for ot in (6, 7)}
                obs = {ot: osb.tile([P, 4, 512], F32, name="ob")
                       for ot in (6, 7)}
                for ts in range(4):
                    for dc in range(KC):
                        for ot in (6, 7):
                            mm(psos[ot][ts], ot, ts, dc,
                               dc == 0, dc == KC - 1)
                    for ot in (6, 7):
                        (nc.scalar.copy if ot == 6 else
                         nc.vector.tensor_copy)(obs[ot][:, ts, :],
                                                psos[ot][ts][:])
                        (nc.sync if ot == 6 else nc.scalar).dma_start(
                            out_d[ts * P:(ts + 1) * P,
                                  ot * 512:(ot + 1) * 512],
                            obs[ot][:, ts, :])

    nc.compile()
    _dedupe_ldweights(nc)
    return nc


# ---------------------------------------------------------------------------
# public entry
# ---------------------------------------------------------------------------

_CACHE = {}


def _execute(x, freqs_cis, mask_cache, input_pos, wqkv, wo,
             trace=False, debug=False):
    _install_profile_shim()
    prep = _prep(x, freqs_cis, mask_cache, wqkv, wo)
    cls = prep['cls']
    n_masks = 0 if prep['masks'] is None else prep['masks'].shape[0] // P
    key = (str(cls), n_masks, debug)
    if key not in _CACHE:
        _CACHE[key] = _build(cls, n_masks, debug=debug)
    nc = _CACHE[key]

    in_maps = []
    for c in range(NCORES):
        m = dict(xT=prep['xT'], wT=prep['wTs'][c], woT=prep['woT'],
                 ropeA=prep['ropeA'], ropeB=prep['ropeB'],
                 ones=prep['ones'], tri=prep['tri'])
        if n_masks:
            m['masks'] = prep['masks']
        in_maps.append(m)

    res = run_bass_kernel_spmd(nc, in_maps, core_ids=list(range(NCORES)),
                               trace=trace,
                               trace_cores=list(range(NCORES)) if trace
                               else None)
    out = np.zeros((B, S, DIM), dtype=np.float32)
    for c in range(NCORES):
        b, j = c // QT_N, c % QT_N
        out[b, j * 512:(j + 1) * 512] = res.results[c]['out']
    return out, res


def kernel(x, freqs_cis, mask_cache, input_pos, wqkv, wo):
    out, _ = _execute(x, freqs_cis, mask_cache, input_pos, wqkv, wo)
    return out


# ---------------------------------------------------------------------------
# numpy simulation of the exact device pipeline (for validation)
# ---------------------------------------------------------------------------

def _simulate(x, freqs_cis, mask_cache, wqkv, wo, use_bf16=True):
    """Mirror the device computation in numpy.  Returns (out, debug_dict)."""
    def q_(a):  # quantize
        return a.astype(bf16).astype(np.float32) if use_bf16 else a

    prep = _prep(x, freqs_cis, mask_cache, wqkv, wo)
    cls = prep['cls']
    xT = prep['xT'].astype(np.float32)
    ropeA = np.concatenate([prep['ropeA'].astype(np.float32)] * B, axis=1)
    ropeB = np.concatenate([prep['ropeB'].astype(np.float32)] * B, axis=1)
    mask = np.asarray(mask_cache)[0, 0]

    dbg = {c: {} for c in range(NCORES)}
    a2a_ins = {m: [] for m in range(HPC)}  # m -> [core][8*128, 512]
    for c in range(NCORES):
        wT = prep['wTs'][c].astype(np.float32)
        qkvT = q_(wT.T @ xT)       # [768, TOK]  (psum f32, evict to bf16)
        sw = np.empty_like(qkvT[:5 * P])
        for f in range(5):
            blk = qkvT[f * P:(f + 1) * P]
            sw[f * P:(f + 1) * P] = q_(blk[[i ^ 1 for i in range(P)], :])
        roped = np.empty_like(qkvT[:5 * P])
        for f in range(5):
            blk = qkvT[f * P:(f + 1) * P]
            r1 = q_(blk * ropeA)
            r2 = q_(sw[f * P:(f + 1) * P] * ropeB)
            roped[f * P:(f + 1) * P] = q_(r1 + r2)
        qTs = [roped[h * P:(h + 1) * P] for h in range(HPC)]
        kTc = roped[4 * P:5 * P]
        vT = qkvT[5 * P:6 * P]     # [128 d, TOK], not roped
        for h in range(HPC):
            a2a_c = np.zeros((NCORES * P, 512), dtype=np.float32)
            for b in range(B):
                kTb = kTc[:, b * S:(b + 1) * S]
                vTb = vT[:, b * S:(b + 1) * S]
                qTb = qTs[h][:, b * S:(b + 1) * S]
                sT = kTb.T @ qTb               # [Sk, Sq] psum f32
                e = q_(np.exp(sT * SCALE))     # ACT exp -> bf16
                emask = e * mask.T             # mask multiply (exact 0/1)
                for qt in range(QT_N):
                    for kt in range(KT_N):
                        if cls[qt][kt] == 'skip':
                            emask[kt * P:(kt + 1) * P,
                                  qt * 512:(qt + 1) * 512] = 0
                # denominator via bf16 pair sums, accumulated in f32
                D = np.zeros(S, dtype=np.float32)
                for qt in range(QT_N):
                    kts = [kt for kt in range(KT_N)
                           if cls[qt][kt] != 'skip']
                    pairs = _denom_pairs(kts, cls, qt)
                    qs = slice(qt * 512, (qt + 1) * 512)
                    for grp in pairs:
                        if grp[0] == 'single':
                            kt = kts[grp[1]]
                            D[qs] += emask[kt * P:(kt + 1) * P, qs].sum(0)
                        else:
                            ka, kb = kts[grp[1]], kts[grp[2]]
                            ps = q_(emask[ka * P:(ka + 1) * P, qs] +
                                    emask[kb * P:(kb + 1) * P, qs])
                            D[qs] += ps.sum(0)
                rec = 1.0 / D
                yTu = vTb @ emask
                y = q_(yTu * rec[None, :])
                for qt in range(QT_N):
                    r = b * QT_N + qt
                    a2a_c[r * P:(r + 1) * P] = y[:, qt * 512:(qt + 1) * 512]
            a2a_ins[h].append(a2a_c)

    # route the A2As:  out shard j on rank c = rank j's input shard c
    out_full = np.zeros((B, S, DIM), dtype=np.float32)
    woT = prep['woT'].astype(np.float32)
    for c in range(NCORES):
        yfull = np.zeros((DIM, 512), dtype=np.float32)
        for m in range(HPC):
            for j in range(NCORES):
                dbi = m * NCORES + j
                yfull[dbi * P:(dbi + 1) * P] = \
                    a2a_ins[m][j][c * P:(c + 1) * P]
        o = yfull.T @ woT          # [512 tok, DIM] psum f32
        b, jj = c // QT_N, c % QT_N
        out_full[b, jj * 512:(jj + 1) * 512] = o
    return out_full, dbg


# revision 28
# speedup vs baseline: 1.0391x; 1.0191x over previous
"""Distributed Trainium2 kernel for nn_Attention_37958920962105.

GQA attention layer (DIM=4096, 32 q heads, 8 kv heads, head_dim=128,
B=2, S=2048) sharded tensor-parallel over GQA groups across 8 cores:
core c owns q heads 4c..4c+3 and kv head c.  Per core:
  1. QKV projection (transposed layouts) + RoPE (pair-swap via strided
     SBUF->SBUF DMA) + v transpose (XBAR DMA transpose)
  2. attention (scores -> exp -> paired-denominator matmuls -> AV)
  3. AllToAll (one per local head) to token-shard y
  4. out projection on this core's 512-token chunk (first two output
     tiles run their m<=2 chunks first so the last AllToAll hides)
Output chunks are reassembled on the host.
"""

import math
import sys
import types
from contextlib import ExitStack

import numpy as np
import ml_dtypes

import concourse.bass as bass
import concourse.mybir as mybir
import concourse.tile as tile
from concourse import bacc
from concourse.bass_utils import run_bass_kernel_spmd

BF = mybir.dt.bfloat16
F32 = mybir.dt.float32
bf16 = ml_dtypes.bfloat16

P = 128
DIM = 4096
N_HEAD = 32
N_KV = 8
HD = 128
B = 2
S = 2048
TOK = B * S          # 4096
NCORES = 8
HPC = N_HEAD // N_KV  # 4 q heads per core
FQKV = (HPC + 2) * HD  # 768 qkv rows per core
KC = DIM // P        # 32 contraction chunks
KCB = 8              # kc chunks per batched DMA
N_TT = TOK // 512    # 8 token tiles of 512
QT_N = S // 512      # 4 q tiles per batch
KT_N = S // P        # 16 k tiles per batch
SCALE = 1.0 / math.sqrt(HD)


def _install_profile_shim():
    if 'antenv.axon_hooks' in sys.modules:
        return
    try:
        from trn_agent_boot.trn_boot import _ntff_profile_via_ctypes
        hook = _ntff_profile_via_ctypes('/opt/axon/libaxon_pjrt.so')
    except Exception:
        hook = None
    mod = types.ModuleType('antenv.axon_hooks')
    mod._hook = hook
    mod.get_axon_ntff_profile_hook = lambda: mod._hook
    mod.set_axon_ntff_profile_hook = lambda h: setattr(mod, '_hook', h)
    sys.modules['antenv.axon_hooks'] = mod
    try:
        import antenv
        antenv.axon_hooks = mod
    except ImportError:
        pass


# ---------------------------------------------------------------------------
# host-side prep
# ---------------------------------------------------------------------------

def _classify_mask(mask):
    """mask: [S(q), S(k)] bool.  Returns (cls, mask_tiles) where
    cls[qt][kt] in {'skip', 'free', int mask-tile-index}; mask tiles are
    transposed [128 k, 512 q] bf16."""
    cls = [[None] * KT_N for _ in range(QT_N)]
    tiles = []
    seen = {}
    qi = np.arange(512)[:, None]
    ki = np.arange(P)[None, :]
    for qt in range(QT_N):
        for kt in range(KT_N):
            blk = mask[qt * 512:(qt + 1) * 512, kt * P:(kt + 1) * P]
            if not blk.any():
                cls[qt][kt] = 'skip'
                continue
            if blk.all():
                cls[qt][kt] = 'free'
                continue
            off = kt * P - qt * 512
            if 0 <= off < 512 and np.array_equal(blk, (off + ki) <= qi):
                # causal diagonal tile: valid only for q >= off, and within
                # the first 128 valid q columns it is the k<=q' triangle
                cls[qt][kt] = ('diag', off)
                continue
            key = blk.tobytes()
            if key not in seen:
                seen[key] = len(tiles)
                tiles.append(np.ascontiguousarray(blk.T).astype(bf16))
            cls[qt][kt] = seen[key]
    return cls, tiles


def _prep(x, freqs_cis, mask_cache, wqkv, wo):
    x = np.asarray(x, dtype=np.float32)
    freqs_cis = np.asarray(freqs_cis, dtype=np.float32)
    wqkv = np.asarray(wqkv, dtype=np.float32)
    wo = np.asarray(wo, dtype=np.float32)
    mask = np.asarray(mask_cache)[0, 0]

    xT = np.ascontiguousarray(x.reshape(TOK, DIM).T).astype(bf16)

    wTs = []
    for c in range(NCORES):
        w_c = np.concatenate([
            wqkv[HPC * HD * c: HPC * HD * (c + 1)],          # 4 q heads
            wqkv[N_HEAD * HD + HD * c: N_HEAD * HD + HD * (c + 1)],   # k head
            wqkv[(N_HEAD + N_KV) * HD + HD * c:
                 (N_HEAD + N_KV) * HD + HD * (c + 1)],       # v head
        ], axis=0)                                           # [768, DIM]
        wTs.append(np.ascontiguousarray(w_c.T).astype(bf16))  # [DIM, 768]

    # wo permuted so row-block dbi = m*8 + cc holds global head 4*cc + m
    woT = np.ascontiguousarray(wo.T)                 # [d, o]
    woT_h = woT.reshape(N_HEAD, HD, DIM)
    perm = [4 * (dbi % NCORES) + dbi // NCORES for dbi in range(N_HEAD)]
    woT_perm = np.ascontiguousarray(woT_h[perm].reshape(DIM, DIM)).astype(bf16)

    f0 = freqs_cis[:, :, 0].T                        # [64, S]
    f1 = freqs_cis[:, :, 1].T
    ropeA = np.repeat(f0, 2, axis=0).astype(bf16)    # [128, S]
    ropeB = np.empty((HD, S), dtype=np.float32)
    ropeB[0::2] = -f1
    ropeB[1::2] = f1
    ropeB = ropeB.astype(bf16)

    ones_col = np.ones((P, 1), dtype=bf16)

    tri = (np.arange(P)[:, None] <= np.arange(P)[None, :]).astype(bf16)
    cls, mask_tiles = _classify_mask(mask)
    masks = (np.concatenate([t for t in mask_tiles], axis=0)
             if mask_tiles else None)                # [n*128, 512] bf16

    return dict(xT=xT, wTs=wTs, woT=woT_perm, ropeA=ropeA, ropeB=ropeB,
                ones=ones_col, tri=tri, cls=cls, masks=masks)


# ---------------------------------------------------------------------------
# device kernel builder
# ---------------------------------------------------------------------------

def _denom_pairs(kts, cls, qt):
    """Group the kt tiles of one (qt) into pairs for the denominator
    matmuls.  Returns a list of ('pair', i, j) / ('single', i) entries
    over indices into kts.  Free tiles are paired in order; the 4 diag
    tiles are paired (w512,w384) and (w256,w128)."""
    free = [i for i, kt in enumerate(kts) if cls[qt][kt] == 'free']
    diag = [i for i, kt in enumerate(kts) if isinstance(cls[qt][kt], tuple)]
    out = []
    for a in range(0, len(free) - 1, 2):
        out.append(('pair', free[a], free[a + 1]))
    if len(free) % 2:
        out.append(('single', free[-1]))
    # diag tiles sorted by offset ascending (w descending)
    diag.sort(key=lambda i: cls[qt][kts[i]][1])
    for a in range(0, len(diag) - 1, 2):
        out.append(('dpair', diag[a], diag[a + 1]))
    if len(diag) % 2:
        out.append(('single', diag[-1]))
    return out


def _dedupe_ldweights(nc):
    """Remove InstLdweights whose weights AP matches the previous PE
    weight load: the PE array already holds those weights, and skipping
    the reload removes the ~128-cycle drain-before-weight-swap stall
    between back-to-back matmuls.  Only drops loads with no semaphore
    waits/updates.  Safe here because every stationary operand in this
    kernel lives in a non-rotating SBUF tile (w_sb, kT, vv, ones, yf)."""
    n_removed = 0
    for fn in nc.m.functions:
        for bb in fn.blocks:
            lst = bb.instructions
            out = []
            prev_sig = None
            changed = False
            for inst in lst:
                nm = type(inst).__name__
                if nm == 'InstLdweights':
                    sig = (str(inst.ins[0]), str(inst.perf_mode),
                           str(inst.is_transpose), str(inst.tile_position))
                    if sig == prev_sig and inst.sync_info is None:
                        n_removed += 1
                        changed = True
                        continue
                    prev_sig = sig
                elif nm == 'InstMatmult':
                    if inst.is_transpose:
                        prev_sig = None
                elif nm in ('InstDrain', 'InstISA', 'InstCall',
                            'InstUnconditionalBranch'):
                    prev_sig = None
                out.append(inst)
            if changed:
                bb.instructions = out
    return n_removed


def _build(cls, n_masks, debug=False):
    nc = bacc.Bacc("TRN2", target_bir_lowering=False, debug=False,
                   num_devices=NCORES)
    xT_d = nc.dram_tensor("xT", [DIM, TOK], BF, kind="ExternalInput")
    wT_d = nc.dram_tensor("wT", [DIM, FQKV], BF, kind="ExternalInput")
    woT_d = nc.dram_tensor("woT", [DIM, DIM], BF, kind="ExternalInput")
    ropeA_d = nc.dram_tensor("ropeA", [P, S], BF, kind="ExternalInput")
    ropeB_d = nc.dram_tensor("ropeB", [P, S], BF, kind="ExternalInput")
    ones_d = nc.dram_tensor("ones", [P, 1], BF, kind="ExternalInput")
    tri_d = nc.dram_tensor("tri", [P, P], BF, kind="ExternalInput")
    masks_d = (nc.dram_tensor("masks", [n_masks * P, 512], BF,
                              kind="ExternalInput") if n_masks else None)
    out_d = nc.dram_tensor("out", [512, DIM], F32, kind="ExternalOutput")

    EXP = mybir.ActivationFunctionType.Exp
    rg = [list(range(NCORES))]

    with tile.TileContext(nc) as tc:
        with ExitStack() as top:
            const = top.enter_context(tc.tile_pool(name="const", bufs=1))
            actsA = top.enter_context(tc.tile_pool(name="actsA", bufs=1))
            actsB = top.enter_context(tc.tile_pool(name="actsB", bufs=1))
            dramp = top.enter_context(tc.tile_pool(name="dramp", bufs=1,
                                                   space="DRAM"))

            qT = [actsA.tile([P, TOK], BF, name=f"qT{h}") for h in range(HPC)]
            kT = actsA.tile([P, TOK], BF, name="kT")
            vv = [actsA.tile([P, P], BF, name=f"v{i}") for i in range(TOK // P)]

            a2a_in = [dramp.tile([NCORES * P, 512], BF, name=f"a2ai{m}")
                      for m in range(HPC)]
            a2a_out = [dramp.tile([NCORES * P, 512], BF, name=f"a2ao{m}")
                       for m in range(HPC)]

            # ---------------- phase 1: QKV + rope + v transpose ----------
            with ExitStack() as ph1, nc.named_scope("ph1_qkv"):
                wp = ph1.enter_context(tc.tile_pool(name="wp", bufs=1))
                xp = ph1.enter_context(tc.tile_pool(name="xp", bufs=6))
                w_src = wT_d[:].rearrange("(kc p) f -> kc p f", p=P)
                x_src = xT_d[:].rearrange("(kc p) t -> kc p t", p=P)

                def load_x_batch(xt, tt, kcb):
                    (nc.scalar if kcb % 2 else nc.sync).dma_start(
                        xt[:], xT_d[:, tt * 512:(tt + 1) * 512].rearrange(
                            "(kcb kc p) t -> kcb p kc t", p=P, kc=KCB)[kcb])

                # fine-grained interleaved first chunks so the very first
                # matmul can start after ~400 KiB instead of 2.5 MiB
                w0 = wp.tile([P, KCB, FQKV], BF, name="w0")
                x0 = xp.tile([P, KCB, 512], BF, name="xt")
                for kc in range(KCB):
                    (nc.sync if kc % 2 == 0 else nc.scalar).dma_start(
                        w0[:, kc, :], w_src[kc])
                    (nc.scalar if kc % 2 == 0 else nc.sync).dma_start(
                        x0[:, kc, :], x_src[kc, :, 0:512])
                w_sb = [w0]
                xts_cur = [x0]
                for kcb in range(1, KC // KCB):
                    w = wp.tile([P, KCB, FQKV], BF, name=f"w{kcb}")
                    (nc.sync if kcb % 2 else nc.scalar).dma_start(
                        w[:], wT_d[:].rearrange(
                            "(kcb kc p) f -> kcb p kc f", p=P, kc=KCB
                        )[kcb])
                    w_sb.append(w)
                    xt = xp.tile([P, KCB, 512], BF, name="xt")
                    load_x_batch(xt, 0, kcb)
                    xts_cur.append(xt)

                def load_x(tt):
                    tiles = []
                    for kcb in range(KC // KCB):
                        xt = xp.tile([P, KCB, 512], BF, name="xt")
                        load_x_batch(xt, tt, kcb)
                        tiles.append(xt)
                    return tiles

                ropeA_sb = wp.tile([P, S], BF, name="ropeA_sb")
                nc.sync.dma_start(ropeA_sb[:], ropeA_d[:])
                ropeB_sb = wp.tile([P, S], BF, name="ropeB_sb")
                nc.sync.dma_start(ropeB_sb[:], ropeB_d[:])
                ones_sb = const.tile([P, 1], BF, name="ones_sb")
                nc.sync.dma_start(ones_sb[:], ones_d[:])
                tri_sb = const.tile([P, P], BF, name="tri_sb")
                nc.sync.dma_start(tri_sb[:], tri_d[:])
                mask_sb = []
                for i in range(n_masks):
                    m = const.tile([P, 512], BF, name=f"mask{i}")
                    nc.sync.dma_start(m[:], masks_d[i * P:(i + 1) * P, :])
                    mask_sb.append(m)
                qkvp = ph1.enter_context(
                    tc.tile_pool(name="qkvp", bufs=8, space="PSUM"))
                stg = ph1.enter_context(tc.tile_pool(name="stg", bufs=4))

                for tt in range(N_TT):
                    s0 = (tt % QT_N) * 512
                    xts = xts_cur
                    if tt + 1 < N_TT:
                        xts_next = load_x(tt + 1)
                    pss = [qkvp.tile([P, 512], F32, name="qkvps")
                           for _ in range(6)]
                    for kcb in range(KC // KCB):
                        for kc in range(KCB):
                            for f in range(6):
                                nc.tensor.matmul(
                                    pss[f][:],
                                    w_sb[kcb][:, kc, f * P:(f + 1) * P],
                                    xts[kcb][:, kc, :],
                                    start=(kcb == 0 and kc == 0),
                                    stop=(kcb == KC // KCB - 1 and
                                          kc == KCB - 1))
                    f_order = (5, 0, 1, 2, 3, 4) if tt == N_TT - 1 \
                        else (0, 1, 2, 3, 4, 5)
                    for fi, f in enumerate(f_order):
                        raw = stg.tile([P, 512], BF, name="raw")
                        # alternate eviction engines so the bank-reuse wait
                        # at the next token tile is halved
                        (nc.scalar.copy if fi % 2 == 0 else
                         nc.vector.tensor_copy)(raw[:], pss[f][:])
                        if f < 5:
                            sw_sb = stg.tile([P, 512], BF, name="sw_sb")
                            nc.sync.dma_start(sw_sb[0::2, :], raw[1::2, :])
                            nc.scalar.dma_start(sw_sb[1::2, :], raw[0::2, :])
                            r1 = stg.tile([P, 512], BF, name="r1")
                            nc.vector.tensor_mul(r1[:], raw[:],
                                                 ropeA_sb[:, s0:s0 + 512])
                            r2 = stg.tile([P, 512], BF, name="r2")
                            nc.vector.tensor_mul(r2[:], sw_sb[:],
                                                 ropeB_sb[:, s0:s0 + 512])
                            dst = (qT[f] if f < HPC else kT)
                            nc.vector.tensor_add(
                                dst[:, tt * 512:(tt + 1) * 512], r1[:], r2[:])
                        else:
                            for j in range(4):
                                nc.sync.dma_start_transpose(
                                    vv[tt * 4 + j][:],
                                    raw[:, j * P:(j + 1) * P])
                    if tt + 1 < N_TT:
                        xts_cur = xts_next

            # yf[m]: [128, cc, 512] token-sharded y rows for head-group m
            # (allocated after ph1 so its SBUF doesn't collide with ph1 pools)
            yf = [actsB.tile([P, NCORES, 512], BF, name=f"yf{m}")
                  for m in range(HPC)]

            # wo prefetch pool opened early so its DMAs can stream during
            # attention; quarter-ot granularity [128, 8, 512] (1 MiB each)
            wop = top.enter_context(tc.tile_pool(name="wop", bufs=11))
            wo_tiles = {}

            def wo_prefetch(ot, qr, eng=None):
                """load wo chunk for output tile ot, dc quarter qr (8 dc)."""
                t = wop.tile([P, KC // 4, 512], BF, name="wo_sb")
                e = eng or (nc.scalar if (ot + qr) % 2 else nc.sync)
                src = woT_d[:, ot * 512:(ot + 1) * 512].rearrange(
                    "(qr dc p) o -> qr p dc o", p=P, dc=KC // 4)[qr]
                e.dma_start(t[:], src)
                wo_tiles[(ot, qr)] = t

            for ot in (0, 1):
                for qr in range(4):
                    wo_prefetch(ot, qr, nc.sync)

            # ---------------- phase 2: attention + A2A -------------------
            with ExitStack() as ph2, nc.named_scope("ph2_attn"):
                sp = ph2.enter_context(
                    tc.tile_pool(name="sp", bufs=3, space="PSUM"))
                yp = ph2.enter_context(
                    tc.tile_pool(name="yp", bufs=3, space="PSUM"))
                dp = ph2.enter_context(
                    tc.tile_pool(name="dp", bufs=2, space="PSUM"))
                ep = ph2.enter_context(tc.tile_pool(name="ep", bufs=8))
                recd = ph2.enter_context(
                    tc.tile_pool(name="recd", bufs=6, space="DRAM"))
                ys = ph2.enter_context(tc.tile_pool(name="ys", bufs=3))
                rp = ph2.enter_context(tc.tile_pool(name="rp", bufs=3))

                def emit_norm(h, b, qt, yu_sb, b_sb):
                    y_sb = ys.tile([P, 512], BF, name="y_sb")
                    nc.vector.tensor_mul(y_sb[:], yu_sb[:], b_sb[:])
                    r = b * QT_N + qt
                    nc.sync.dma_start(
                        a2a_in[h][r * P:(r + 1) * P, :], y_sb[:])

                yf_pending = []

                def flush_yf():
                    # issued from the gpsimd queue: it already serializes on
                    # the collective, so the wait on the A2A-done semaphore
                    # cannot head-of-line-block the sync DMA queue (which
                    # carries the reciprocal-broadcast chain)
                    while yf_pending:
                        hh = yf_pending.pop(0)
                        nc.gpsimd.dma_start(
                            yf[hh][:],
                            a2a_out[hh][:].rearrange(
                                "(cc p) t -> p cc t", p=P))

                # heads processed in pairs: the two heads at one (b,qt,kt)
                # share the kT / vv / ones stationary operands, so after
                # LDWEIGHTS dedupe every second matmul skips the reload
                for heads in ((0, 2), (1, 3)):
                    pending = []
                    flush_yf()
                    for b in range(B):
                        for qt in range(QT_N):
                            kts = [kt for kt in range(KT_N)
                                   if cls[qt][kt] != 'skip']
                            pairs = _denom_pairs(kts, cls, qt)
                            ps_y = {h: yp.tile([P, 512], F32, name="psy")
                                    for h in heads}
                            ps_d = {h: dp.tile([1, 512], F32, name="psd")
                                    for h in heads}
                            es = {h: [None] * len(kts) for h in heads}
                            offs = [0] * len(kts)

                            def do_tile(i):
                                """scores (both heads) + exp + mask."""
                                kt = kts[i]
                                c = cls[qt][kt]
                                off = c[1] if isinstance(c, tuple) else 0
                                offs[i] = off
                                w_q = 512 - off
                                q0 = b * S + qt * 512 + off
                                pss = {}
                                for h in heads:
                                    ps_s = sp.tile([P, 512], F32,
                                                   name="pss")
                                    nc.tensor.matmul(
                                        ps_s[:, :w_q],
                                        kT[:, b * S + kt * P:
                                           b * S + (kt + 1) * P],
                                        qT[h][:, bass.ds(q0, w_q)],
                                        start=True, stop=True)
                                    pss[h] = ps_s
                                for h in heads:
                                    e = ep.tile([P, 512], BF, name="e")
                                    nc.scalar.activation(
                                        e[:, :w_q], pss[h][:, :w_q], EXP,
                                        scale=SCALE)
                                    if isinstance(c, tuple):
                                        nc.vector.tensor_mul(
                                            e[:, :P], e[:, :P], tri_sb[:])
                                    elif c != 'free':
                                        nc.vector.tensor_mul(
                                            e[:, :512], e[:, :512],
                                            mask_sb[c][:])
                                    es[h][i] = e

                            def do_av(i, first, last):
                                kt = kts[i]
                                off = offs[i]
                                for h in heads:
                                    nc.tensor.matmul(
                                        ps_y[h][:, off:512],
                                        vv[b * KT_N + kt][:],
                                        es[h][i][:, :512 - off],
                                        start=first, stop=last,
                                        skip_group_check=True)

                            def do_denom(rhs_f, off, first, last):
                                for h in heads:
                                    nc.tensor.matmul(
                                        ps_d[h][:, off:512], ones_sb[:],
                                        rhs_f(h)[:, :512 - off],
                                        start=first, stop=last,
                                        skip_group_check=True)

                            n_done = 0
                            n_total = len(kts)
                            first_d = True
                            for gi, grp in enumerate(pairs):
                                last_grp = (gi == len(pairs) - 1)
                                if grp[0] == 'single':
                                    i = grp[1]
                                    do_tile(i)
                                    do_av(i, n_done == 0,
                                          n_done + 1 == n_total)
                                    n_done += 1
                                    do_denom(lambda h: es[h][i], offs[i],
                                             first_d, last_grp)
                                else:
                                    ia, ib = grp[1], grp[2]
                                    do_tile(ia)
                                    do_tile(ib)
                                    do_av(ia, n_done == 0, False)
                                    do_av(ib, False,
                                          n_done + 2 == n_total)
                                    n_done += 2
                                    if grp[0] == 'pair':
                                        ss = {}
                                        for h in heads:
                                            s = ep.tile([P, 512], BF,
                                                        name="es")
                                            nc.vector.tensor_add(
                                                s[:], es[h][ia][:],
                                                es[h][ib][:])
                                            ss[h] = s
                                        do_denom(lambda h: ss[h], 0,
                                                 first_d, last_grp)
                                    else:
                                        # diag pair: fold the narrower
                                        # tile into the wider one
                                        oa, ob = offs[ia], offs[ib]
                                        w_b = 512 - ob
                                        for h in heads:
                                            nc.vector.tensor_add(
                                                es[h][ia][:, ob - oa:
                                                          ob - oa + w_b],
                                                es[h][ia][:, ob - oa:
                                                          ob - oa + w_b],
                                                es[h][ib][:, :w_b])
                                        do_denom(lambda h: es[h][ia], oa,
                                                 first_d, last_grp)
                                first_d = False
                            while len(pending) > 1:
                                emit_norm(*pending.pop(0))
                            for h in heads:
                                yu_sb = ys.tile([P, 512], F32,
                                                name="yu_sb")
                                nc.vector.tensor_copy(yu_sb[:],
                                                      ps_y[h][:])
                                rec = rp.tile([1, 512], F32, name="rec")
                                nc.vector.reciprocal_approx_fast(
                                    rec[:], ps_d[h][:])
                                rec_dram = recd.tile([1, 512], F32,
                                                     name="rec_dram")
                                nc.sync.dma_start(rec_dram[:], rec[:])
                                b_sb = ys.tile([P, 512], F32, name="b_sb")
                                nc.sync.dma_start(
                                    b_sb[:],
                                    rec_dram[:].partition_broadcast(P))
                                pending.append((h, b, qt, yu_sb, b_sb))
                    for pn in pending:
                        emit_norm(*pn)
                    pending = []
                    for h in heads:
                        nc.gpsimd.collective_compute(
                            "AllToAll", mybir.AluOpType.bypass,
                            replica_groups=rg,
                            ins=[a2a_in[h].opt()], outs=[a2a_out[h].opt()])
                        yf_pending.append(h)
                flush_yf()

            # ---------------- phase 3: out projection --------------------
            # ot pairs with dc-outer / ts-inner ordering: the (ot, ot+1)
            # matmuls at one (dc, ts) share the yf stationary chunk (LDW
            # dedupe), wo quarters free progressively as dc advances, and
            # the 8 psum accumulators use all banks
            with ExitStack() as ph3, nc.named_scope("ph3_outp"):
                opp = ph3.enter_context(
                    tc.tile_pool(name="opp", bufs=8, space="PSUM"))
                osb = ph3.enter_context(tc.tile_pool(name="osb", bufs=2))

                def mm(pso, ot, ts, dc, start, stop):
                    nc.tensor.matmul(
                        pso[:], yf[dc // NCORES][:, dc % NCORES,
                                                 ts * P:(ts + 1) * P],
                        wo_tiles[(ot, dc // 8)][:, dc % 8, :],
                        start=start, stop=stop, skip_group_check=True)

                prefetch_q = [(ot, qr) for ot in range(2, 8)
                              for qr in range(4)]

                def evict(ot, psos):
                    ob = osb.tile([P, 4, 512], F32, name="ob")
                    for ts in range(4):
                        (nc.scalar.copy if ts % 2 == 0 else
                         nc.vector.tensor_copy)(ob[:, ts, :], psos[ts][:])
                    nc.sync.dma_start(
                        out_d[:, ot * 512:(ot + 1) * 512].rearrange(
                            "(ts p) o -> p ts o", p=P),
                        ob[:])
                    for qr in range(4):
                        del wo_tiles[(ot, qr)]
                    for _ in range(4):
                        if prefetch_q:
                            wo_prefetch(*prefetch_q.pop(0))

                def run_pair(ota, otb, dc_ranges):
                    psos = {ot: [opp.tile([P, 512], F32, name="pso")
                                 for _ in range(4)] for ot in (ota, otb)}
                    for dcs in dc_ranges:
                        for dc in dcs:
                            for ts in range(4):
                                for ot in (ota, otb):
                                    mm(psos[ot][ts], ot, ts, dc,
                                       dc == 0, dc == KC - 1)
                    evict(ota, psos[ota])
                    evict(otb, psos[otb])

                # first pair: dc 0..23 (m<=2) before dc 24..31 so the last
                # AllToAlls hide under ~45us of independent matmuls
                run_pair(0, 1, (list(range(0, 8)) + list(range(16, 24)),
                                list(range(8, 16)) + list(range(24, KC))))
                run_pair(2, 3, (range(KC),))
                run_pair(4, 5, (range(KC),))
                # last pair: ts-outer so accumulators finish staggered and
                # the final evictions overlap the remaining matmuls
                psos = {ot: [opp.tile([P, 512], F32, name="pso")
                             for _ in range(4)] for ot in (6, 7)}
                obs = {ot: osb.tile([P, 4, 512], F32, name="ob")
                       for ot in (6, 7)}
                for ts in range(4):
                    for dc in range(KC):
                        for ot in (6, 7):
                            mm(psos[ot][ts], ot, ts, dc,
                               dc == 0, dc == KC - 1)
                    for ot in (6, 7):
                        (nc.scalar.copy if ot == 6 else
                         nc.vector.tensor_copy)(obs[ot][:, ts, :],
                                                psos[ot][ts][:])
                        (nc.sync if ot == 6 else nc.scalar).dma_start(
                            out_d[ts * P:(ts + 1) * P,
                                  ot * 512:(ot + 1) * 512],
                            obs[ot][:, ts, :])

    nc.compile()
    _dedupe_ldweights(nc)
    return nc


# ---------------------------------------------------------------------------
# public entry
# ---------------------------------------------------------------------------

_CACHE = {}


def _execute(x, freqs_cis, mask_cache, input_pos, wqkv, wo,
             trace=False, debug=False):
    _install_profile_shim()
    prep = _prep(x, freqs_cis, mask_cache, wqkv, wo)
    cls = prep['cls']
    n_masks = 0 if prep['masks'] is None else prep['masks'].shape[0] // P
    key = (str(cls), n_masks, debug)
    if key not in _CACHE:
        _CACHE[key] = _build(cls, n_masks, debug=debug)
    nc = _CACHE[key]

    in_maps = []
    for c in range(NCORES):
        m = dict(xT=prep['xT'], wT=prep['wTs'][c], woT=prep['woT'],
                 ropeA=prep['ropeA'], ropeB=prep['ropeB'],
                 ones=prep['ones'], tri=prep['tri'])
        if n_masks:
            m['masks'] = prep['masks']
        in_maps.append(m)

    res = run_bass_kernel_spmd(nc, in_maps, core_ids=list(range(NCORES)),
                               trace=trace,
                               trace_cores=list(range(NCORES)) if trace
                               else None)
    out = np.zeros((B, S, DIM), dtype=np.float32)
    for c in range(NCORES):
        b, j = c // QT_N, c % QT_N
        out[b, j * 512:(j + 1) * 512] = res.results[c]['out']
    return out, res


def kernel(x, freqs_cis, mask_cache, input_pos, wqkv, wo):
    out, _ = _execute(x, freqs_cis, mask_cache, input_pos, wqkv, wo)
    return out


# ---------------------------------------------------------------------------
# numpy simulation of the exact device pipeline (for validation)
# ---------------------------------------------------------------------------

def _simulate(x, freqs_cis, mask_cache, wqkv, wo, use_bf16=True):
    """Mirror the device computation in numpy.  Returns (out, debug_dict)."""
    def q_(a):  # quantize
        return a.astype(bf16).astype(np.float32) if use_bf16 else a

    prep = _prep(x, freqs_cis, mask_cache, wqkv, wo)
    cls = prep['cls']
    xT = prep['xT'].astype(np.float32)
    ropeA = np.concatenate([prep['ropeA'].astype(np.float32)] * B, axis=1)
    ropeB = np.concatenate([prep['ropeB'].astype(np.float32)] * B, axis=1)
    mask = np.asarray(mask_cache)[0, 0]

    dbg = {c: {} for c in range(NCORES)}
    a2a_ins = {m: [] for m in range(HPC)}  # m -> [core][8*128, 512]
    for c in range(NCORES):
        wT = prep['wTs'][c].astype(np.float32)
        qkvT = q_(wT.T @ xT)       # [768, TOK]  (psum f32, evict to bf16)
        sw = np.empty_like(qkvT[:5 * P])
        for f in range(5):
            blk = qkvT[f * P:(f + 1) * P]
            sw[f * P:(f + 1) * P] = q_(blk[[i ^ 1 for i in range(P)], :])
        roped = np.empty_like(qkvT[:5 * P])
        for f in range(5):
            blk = qkvT[f * P:(f + 1) * P]
            r1 = q_(blk * ropeA)
            r2 = q_(sw[f * P:(f + 1) * P] * ropeB)
            roped[f * P:(f + 1) * P] = q_(r1 + r2)
        qTs = [roped[h * P:(h + 1) * P] for h in range(HPC)]
        kTc = roped[4 * P:5 * P]
        vT = qkvT[5 * P:6 * P]     # [128 d, TOK], not roped
        for h in range(HPC):
            a2a_c = np.zeros((NCORES * P, 512), dtype=np.float32)
            for b in range(B):
                kTb = kTc[:, b * S:(b + 1) * S]
                vTb = vT[:, b * S:(b + 1) * S]
                qTb = qTs[h][:, b * S:(b + 1) * S]
                sT = kTb.T @ qTb               # [Sk, Sq] psum f32
                e = q_(np.exp(sT * SCALE))     # ACT exp -> bf16
                emask = e * mask.T             # mask multiply (exact 0/1)
                for qt in range(QT_N):
                    for kt in range(KT_N):
                        if cls[qt][kt] == 'skip':
                            emask[kt * P:(kt + 1) * P,
                                  qt * 512:(qt + 1) * 512] = 0
                # denominator via bf16 pair sums, accumulated in f32
                D = np.zeros(S, dtype=np.float32)
                for qt in range(QT_N):
                    kts = [kt for kt in range(KT_N)
                           if cls[qt][kt] != 'skip']
                    pairs = _denom_pairs(kts, cls, qt)
                    qs = slice(qt * 512, (qt + 1) * 512)
                    for grp in pairs:
                        if grp[0] == 'single':
                            kt = kts[grp[1]]
                            D[qs] += emask[kt * P:(kt + 1) * P, qs].sum(0)
                        else:
                            ka, kb = kts[grp[1]], kts[grp[2]]
                            ps = q_(emask[ka * P:(ka + 1) * P, qs] +
                                    emask[kb * P:(kb + 1) * P, qs])
                            D[qs] += ps.sum(0)
                rec = 1.0 / D
                yTu = vTb @ emask
                y = q_(yTu * rec[None, :])
                for qt in range(QT_N):
                    r = b * QT_N + qt
                    a2a_c[r * P:(r + 1) * P] = y[:, qt * 512:(qt + 1) * 512]
            a2a_ins[h].append(a2a_c)

    # route the A2As:  out shard j on rank c = rank j's input shard c
    out_full = np.zeros((B, S, DIM), dtype=np.float32)
    woT = prep['woT'].astype(np.float32)
    for c in range(NCORES):
        yfull = np.zeros((DIM, 512), dtype=np.float32)
        for m in range(HPC):
            for j in range(NCORES):
                dbi = m * NCORES + j
                yfull[dbi * P:(dbi + 1) * P] = \
                    a2a_ins[m][j][c * P:(c + 1) * P]
        o = yfull.T @ woT          # [512 tok, DIM] psum f32
        b, jj = c // QT_N, c % QT_N
        out_full[b, jj * 512:(jj + 1) * 512] = o
    return out_full, dbg
